# revision 18
# baseline (speedup 1.0000x reference)
"""Trainium2 Bass kernel for nn_DressedQuantumNet.

Math reformulation (exact, up to float rounding):
  pre_out = x @ pre_w.T + pre_b                  # [B,4]
  theta_w = (pi/4)*tanh(pre_out_w) + pi/4        # in (0, pi/2)
  v_w     = [cos theta_w, sin theta_w]           # per-qubit state (positive)
  psi     = v_0 (x) v_1 (x) v_2 (x) v_3          # [B,16] product state
  phi     = M @ psi        # M = fixed 16x16 matrix of the CNOT/RY circuit
  out     = (phi*phi)^T P + post_b  # P[i,c] = sum_w post_w[c,w] * z_w(i)

Device strategy (pure data parallel over 8 cores, 8192 samples each):
  - x is downcast to bf16 AND transposed on host, so the device streams
    xT with plain, fully-contiguous DMA (4KB descriptor lines) at full
    HBM rate -- no DMA-xbar transposes anywhere in the kernel.
  - pre-matmul: lhsT = tiny pre_w chunk [128d, 4] (stationary), rhs = xT
    block [128d, 512 samples] streaming at 1 col/cycle bf16, accumulated
    over 4 d-chunks into PSUM; two groups share a [4, 1024] PSUM tile so
    one fused bias+tanh (ScalarE) covers 1024 samples.
  - back to sample-major via tiny PE transposes: t4 blocks [4, 128] ->
    thT PSUM [128, 4] (contraction k=4, so no pad rows are ever touched);
    they pipeline at ~32ns apiece on the PE exec queue, batched 16 tiles
    per PSUM tile with one DVE copy per 2048-sample chunk.
  - trig on ScalarE: 2x Sin with scale/bias folding cos, outputs packed
    as [cos(t,w) | sin(t,w)] so the kron products use x-strided APs.
  - psi built with 3 broadcast-AP vector multiplies (bf16 out).
  - quantum circuit in bf16: PE transpose psi -> [16 comps x 8 tiles,
    samples], then block-diagonal M (16x16 circuit matrix) and P
    (measurement x post_w) matmuls with 256-col moving operands.
  - square and +post_b run on DVE; per-chunk output stores overlap.
"""

import os
import sys

for _p in ("/opt/trn_rl_repo",):
    if os.path.isdir(_p) and _p not in sys.path:
        sys.path.insert(0, _p)

import math
import numpy as np
import ml_dtypes
from contextlib import ExitStack

import concourse.bass as bass
import concourse.bacc as bacc
import concourse.mybir as mybir
from concourse.tile import TileContext
from concourse.bass_utils import run_bass_kernel_spmd

F32 = mybir.dt.float32
BF16 = mybir.dt.bfloat16
AF = mybir.ActivationFunctionType
PI4 = math.pi / 4.0

N_CORES = 8
B_FULL, D, C = 65536, 512, 10
B = B_FULL // N_CORES          # 8192 samples per core
N_QUBITS, Q_DEPTH = 4, 6
TILES = B // 128               # 64 sample tiles of 128
BLOCKS = 4                     # x load blocks of 2048 samples (4KB lines)
CHUNKS = 4                     # phase-2 chunks of 2048 samples (16 tiles)

# packed bf16 const block column offsets
CB_PREWT = 0      # [128, 16]
CB_MBD = 16       # [128, 128]
CB_PBD = 144      # [128, 80]
CB_ID = 224       # [128, 128]
CB_N = 352


# ---------------------------------------------------------------- host math
def _apply_1q(state, gate, wire):
    state = np.moveaxis(state, wire, 0)
    state = np.tensordot(gate, state, axes=((1,), (0,)))
    return np.moveaxis(state, 0, wire)


def _apply_cnot(state, ctrl, tgt):
    state = np.moveaxis(state, (ctrl, tgt), (0, 1))
    state = np.stack([state[0], state[1][::-1]], axis=0)
    return np.moveaxis(state, (0, 1), (ctrl, tgt))


def _ry(theta):
    c, s = np.cos(theta * 0.5), np.sin(theta * 0.5)
    return np.array([[c, -s], [s, c]])


def _build_M(q_params: np.ndarray) -> np.ndarray:
    """16x16 matrix of the fixed part of the circuit (after the per-sample
    RY layer): 6 repetitions of [CNOT(0,1), CNOT(2,3), CNOT(1,2), RY layer]."""
    qw = np.asarray(q_params, np.float64).reshape(Q_DEPTH, N_QUBITS)
    M = np.zeros((16, 16), np.float64)
    for i in range(16):
        state = np.zeros(16, np.float64)
        state[i] = 1.0
        state = state.reshape((2,) * N_QUBITS)
        for k in range(Q_DEPTH):
            for a in range(0, N_QUBITS - 1, 2):
                state = _apply_cnot(state, a, a + 1)
            for a in range(1, N_QUBITS - 1, 2):
                state = _apply_cnot(state, a, a + 1)
            for w in range(N_QUBITS):
                state = _apply_1q(state, _ry(qw[k, w]), w)
        M[:, i] = state.reshape(16)
    return M


def _build_P(post_w: np.ndarray) -> np.ndarray:
    """P[i, c] = sum_w post_w[c, w] * z_w(i), where z_w(i) flips sign with
    bit (3-w) of the state index i (axis 0 of the state = qubit 0)."""
    post_w = np.asarray(post_w, np.float64)
    i = np.arange(16)
    z = np.stack([1.0 - 2.0 * ((i >> (3 - w)) & 1) for w in range(N_QUBITS)], 1)
    return z @ post_w.T  # [16, 10]


# ---------------------------------------------------------------- bass build
def build_nc(sim_compat: bool = False) -> bass.Bass:
    # Bacc (not raw Bass): its finalize() runs generate_event_semaphores,
    # which splits multi-semaphore waits to satisfy the TRN2 one-wait-per-
    # instruction ISA limit.
    nc = bacc.Bacc(None)
    xt = nc.dram_tensor("xt", [D, B], BF16, kind="ExternalInput")
    cbf = nc.dram_tensor("cbf", [128, CB_N], BF16, kind="ExternalInput")
    cf32 = nc.dram_tensor("cf32", [128, 4], F32, kind="ExternalInput")
    # transposed on device: out[tile, class, sample-in-tile]; host flips back
    out = nc.dram_tensor("out", [TILES, C, 128], F32, kind="ExternalOutput")

    with ExitStack() as ctx:
        tc = ctx.enter_context(TileContext(nc))
        consts = ctx.enter_context(tc.tile_pool(name="consts", bufs=1))
        # all x block tiles stay resident (8 MB)
        xt_pool = ctx.enter_context(tc.tile_pool(name="xt", bufs=20))
        work = ctx.enter_context(tc.tile_pool(name="work", bufs=3))
        ps_po = ctx.enter_context(tc.tile_pool(name="ps_po", space="PSUM", bufs=2))
        ps_th = ctx.enter_context(tc.tile_pool(name="ps_th", space="PSUM", bufs=1))
        ps_pt = ctx.enter_context(tc.tile_pool(name="ps_pt", space="PSUM", bufs=1))
        ps_mm = ctx.enter_context(tc.tile_pool(name="ps_mm", space="PSUM", bufs=2))

        # consts go on the sync queue BEFORE the x loads so their (tiny)
        # transfers land first and the PE warmup can start early
        cbf_sb = consts.tile([128, CB_N], BF16)
        nc.sync.dma_start(cbf_sb, cbf[:, :])
        cf32_sb = consts.tile([128, 4], F32)
        nc.sync.dma_start(cf32_sb, cf32[:, :])

        pre_wt_sb = cbf_sb[:, CB_PREWT:CB_PREWT + 16]
        mbd_sb = cbf_sb[:, CB_MBD:CB_MBD + 128]
        pbd_sb = cbf_sb[:, CB_PBD:CB_PBD + 80]
        id_sb = cbf_sb[:, CB_ID:CB_ID + 128]
        id4_sb = cbf_sb[0:4, CB_ID:CB_ID + 4]
        pre_b_sb = cf32_sb[0:4, 0:1]
        pb80_sb = cf32_sb[0:80, 1:2]
        trigb_sb = cf32_sb[:, 2:4]

        # tanh staging, component-major: t4[f, b] = tanh(pre_out[b, f])
        t4_sb = consts.tile([4, B], BF16)
        # transposed output staging [80 = 8 tiles x 10 classes, 1024]
        out2_sb = consts.tile([80, 256 * CHUNKS], F32)

        # pin the activation table to silu_and_others once: it is the only
        # table containing tanh+sin+square+identity together, so no further
        # table loads happen. (CoreSim can't evaluate Silu; the sim build
        # substitutes Tanh -- the value is unused either way.)
        silu_sb = consts.tile([128, 1], F32)
        nc.scalar.activation(silu_sb, cf32_sb[:, 0:1],
                             AF.Tanh if sim_compat else AF.Silu)

        # ---- all x loads up front: plain contiguous DMA on the SP queue,
        # one per (d-chunk, 2048-sample block) with 4KB descriptor lines
        xtt = []  # xtt[b][k] = [128 (d of chunk k), 2048 samples]
        for b in range(BLOCKS):
            tiles_k = []
            for k in range(4):
                t = xt_pool.tile([128, 2048], BF16, name=f"xt{b}_{k}", tag="xt")
                nc.sync.dma_start(
                    t, xt[128 * k:128 * (k + 1), 2048 * b:2048 * (b + 1)])
                tiles_k.append(t)
            xtt.append(tiles_k)

        # PE p-state warmup while the first loads are in flight (results
        # unused; depends only on the cbf const load, which lands first)
        for w in range(4):
            wt = ps_pt.tile([128, 128], BF16, name=f"warm{w}", tag="pt")
            nc.tensor.transpose(wt, id_sb, id_sb)

        def phase1(b):
            # pre-net: two 512-groups share one [4,1024] PSUM tile (two
            # accumulation groups in adjacent banks) so a single fused
            # bias+tanh (ScalarE) covers 1024 samples
            for uu in range(2):
                po = ps_po.tile([4, 1024], F32, name="po", tag="po")
                for j in range(2):
                    gl = 2 * uu + j
                    for k in range(4):
                        nc.tensor.matmul(
                            po[:, 512 * j:512 * (j + 1)],
                            lhsT=pre_wt_sb[:, 4 * k:4 * (k + 1)],
                            rhs=xtt[b][k][:, 512 * gl:512 * (gl + 1)],
                            start=(k == 0), stop=(k == 3))
                nc.scalar.activation(
                    t4_sb[:, 2048 * b + 1024 * uu:2048 * b + 1024 * (uu + 1)],
                    po, AF.Tanh, bias=pre_b_sb[:, :])

        def phase2(T0, nt):
            # chunk = nt sample-tiles starting at tile T0 (nt = 8 or 16).
            # sample-major angles via nt tiny PE transposes (k=4
            # contraction, so only the 4 real rows of t4 are read); the
            # Sin ops read the batched transpose tile straight from PSUM
            nh = nt // 8
            thT_ps = ps_th.tile([128, 4 * nt], BF16, name="thT_ps", tag="th")
            for t in range(nt):
                T = T0 + t
                nc.tensor.transpose(
                    thT_ps[:, 4 * t:4 * (t + 1)],
                    t4_sb[:, 128 * T:128 * (T + 1)], id4_sb)
            th3 = thT_ps[:, :].rearrange("p (t i) -> p t i", i=4)
            # cs2 packed: cols 0:4nt = cos(theta)[t,w], 4nt:8nt = sin[t,w]
            # cos = sin(pi/4*tanh + 3pi/4); sin = sin(.. + pi/4)
            cs2 = work.tile([128, 8 * nt], F32, name="cs2", tag="cs2")
            nc.scalar.activation(
                cs2[:, 0:4 * nt].rearrange("p (t w) -> p t w", w=4), th3,
                AF.Sin, bias=trigb_sb[:, 0:1], scale=PI4)
            nc.scalar.activation(
                cs2[:, 4 * nt:8 * nt].rearrange("p (t w) -> p t w", w=4), th3,
                AF.Sin, bias=trigb_sb[:, 1:2], scale=PI4)
            ctxw = cs2[:, :].rearrange("p (x t w) -> p t x w", x=2, w=4)
            ctwx = cs2[:, :].rearrange("p (x t w) -> p t w x", x=2, w=4)
            v01 = work.tile([128, 4 * nt], F32, name="v01", tag="v01")
            v23 = work.tile([128, 4 * nt], F32, name="v23", tag="v23")
            nc.vector.tensor_tensor(
                out=v01[:, :].rearrange("p (t a b) -> p t a b", a=2, b=2),
                in0=ctxw[:, :, :, 0:1].broadcast_to((128, nt, 2, 2)),
                in1=ctwx[:, :, 1:2, :].broadcast_to((128, nt, 2, 2)),
                op=mybir.AluOpType.mult)
            nc.vector.tensor_tensor(
                out=v23[:, :].rearrange("p (t a b) -> p t a b", a=2, b=2),
                in0=ctxw[:, :, :, 2:3].broadcast_to((128, nt, 2, 2)),
                in1=ctwx[:, :, 3:4, :].broadcast_to((128, nt, 2, 2)),
                op=mybir.AluOpType.mult)
            psi = work.tile([128, 16 * nt], BF16, name="psi", tag="psi")
            nc.vector.tensor_tensor(
                out=psi[:, :].rearrange("p (t a b) -> p t a b", a=4, b=4),
                in0=v01[:, :].rearrange("p (t i) -> p t i", i=4)
                    .unsqueeze(3).broadcast_to((128, nt, 4, 4)),
                in1=v23[:, :].rearrange("p (t i) -> p t i", i=4)
                    .unsqueeze(2).broadcast_to((128, nt, 4, 4)),
                op=mybir.AluOpType.mult)

            # quantum circuit, all bf16: psi^T per h-half, then block-diag
            # M and P matmuls over all halves at once
            psiT_ps = ps_pt.tile([128, 128 * nh], BF16, name="psiT_ps",
                                 tag="pt")
            for h in range(nh):
                nc.tensor.transpose(
                    psiT_ps[:, 128 * h:128 * (h + 1)],
                    psi[:, 128 * h:128 * (h + 1)], id_sb)
            psiT = work.tile([128, 128 * nh], BF16, name="psiT", tag="psiT")
            nc.vector.tensor_copy(psiT, psiT_ps)
            phi_ps = ps_mm.tile([128, 128 * nh], F32, name="phi_ps", tag="mm")
            nc.tensor.matmul(phi_ps, lhsT=mbd_sb, rhs=psiT,
                             start=True, stop=True)
            phi2 = work.tile([128, 128 * nh], BF16, name="phi2", tag="phi2")
            nc.scalar.activation(phi2, phi_ps, AF.Square)
            o10_ps = ps_mm.tile([80, 128 * nh], F32, name="o10_ps", tag="mm")
            nc.tensor.matmul(o10_ps, lhsT=pbd_sb, rhs=phi2,
                             start=True, stop=True)
            # bias-add on DVE (per-partition scalar operand) into the
            # transposed output staging tile
            nc.vector.tensor_scalar(
                out2_sb[:, 16 * T0:16 * T0 + 128 * nh], o10_ps,
                pb80_sb[:, :], None, mybir.AluOpType.add)

            # per-chunk store (plain, on the sync queue behind the loads):
            # out[T0 + 8h + t, cls, j] = out2_sb[10t + cls, 16*T0 + 128h + j]
            nc.sync.dma_start(
                out[T0:T0 + nt, :, :]
                    .rearrange("(h t) c j -> (t c) h j", h=nh),
                out2_sb[:, 16 * T0:16 * T0 + 128 * nh]
                    .rearrange("p (h j) -> p h j", h=nh))

        # software-pipelined emission: each engine's in-order queue gets
        # the next block's independent phase-1 work before the previous
        # chunk's dependent phase-2 chain, so no engine idles inside the
        # cross-engine chain.  The last block is split into two 8-tile
        # chunks to halve the serial drain after the final load.
        chunks = [(0, 16), (16, 16), (32, 16), (48, 16)]
        emitted = 0
        for b in range(BLOCKS):
            phase1(b)
            # emit phase-2 only for chunks covered by the PREVIOUS block,
            # keeping one block of independent phase-1 work ahead of every
            # phase-2 chain in each engine queue
            while emitted < len(chunks) and \
                    chunks[emitted][0] + chunks[emitted][1] <= 16 * b:
                phase2(*chunks[emitted])
                emitted += 1
        while emitted < len(chunks):
            phase2(*chunks[emitted])
            emitted += 1

    nc.finalize()  # bacc: register alloc + event-semaphore wait splitting
    return nc


_NC_CACHE: dict = {}


def _get_nc() -> bass.Bass:
    if "nc" not in _NC_CACHE:
        _NC_CACHE["nc"] = build_nc()
    return _NC_CACHE["nc"]


def make_in_maps(inputs: dict) -> list:
    x = np.asarray(inputs["input_features"], np.float32)
    pre_w = np.asarray(inputs["pre_w"], np.float32)
    pre_b = np.asarray(inputs["pre_b"], np.float32)
    q_params = np.asarray(inputs["q_params"], np.float32)
    post_w = np.asarray(inputs["post_w"], np.float32)
    post_b = np.asarray(inputs["post_b"], np.float32)

    M = _build_M(q_params)
    P = _build_P(post_w)
    mbd = np.zeros((128, 128), np.float32)
    pbd = np.zeros((128, 80), np.float32)
    for t in range(8):
        mbd[16 * t:16 * (t + 1), 16 * t:16 * (t + 1)] = M.T
        pbd[16 * t:16 * (t + 1), 10 * t:10 * (t + 1)] = P
    # pre_wt_sb[p, 4k+f] = pre_w[f, 128k+p]
    pre_wt = np.ascontiguousarray(
        pre_w.T.reshape(4, 128, 4).transpose(1, 0, 2).reshape(128, 16))

    cbf = np.zeros((128, CB_N), np.float32)
    cbf[:, CB_PREWT:CB_PREWT + 16] = pre_wt
    cbf[:, CB_MBD:CB_MBD + 128] = mbd
    cbf[:, CB_PBD:CB_PBD + 80] = pbd
    cbf[:, CB_ID:CB_ID + 128] = np.eye(128, dtype=np.float32)
    cbf = cbf.astype(ml_dtypes.bfloat16)

    cf32 = np.zeros((128, 4), np.float32)
    cf32[0:4, 0] = pre_b
    cf32[0:80, 1] = np.tile(post_b, 8)
    cf32[:, 2] = 3.0 * PI4
    cf32[:, 3] = PI4

    xb = x.astype(ml_dtypes.bfloat16)
    consts = dict(cbf=cbf, cf32=cf32)
    return [dict(xt=np.ascontiguousarray(xb[B * i:B * (i + 1)].T), **consts)
            for i in range(N_CORES)]


def unpack_out(dev_out: np.ndarray) -> np.ndarray:
    """[TILES, C, 128] device layout -> [B, C]."""
    return dev_out.transpose(0, 2, 1).reshape(B, C)


def run_on_device(inputs: dict, **kwargs):
    """Returns (full_output, BassKernelResults)."""
    nc = _get_nc()
    in_maps = make_in_maps(inputs)
    res = run_bass_kernel_spmd(nc, in_maps, core_ids=list(range(N_CORES)),
                               **kwargs)
    full = np.concatenate(
        [unpack_out(res.results[i]["out"]) for i in range(N_CORES)], 0)
    return np.ascontiguousarray(full, dtype=np.float32), res


def kernel(**inputs) -> np.ndarray:
    out, _ = run_on_device(inputs)
    return out


# revision 19
# speedup vs baseline: 1.0351x; 1.0351x over previous
"""Trainium2 Bass kernel for nn_DressedQuantumNet.

Math reformulation (exact, up to float rounding):
  pre_out = x @ pre_w.T + pre_b                  # [B,4]
  theta_w = (pi/4)*tanh(pre_out_w) + pi/4        # in (0, pi/2)
  v_w     = [cos theta_w, sin theta_w]           # per-qubit state (positive)
  psi     = v_0 (x) v_1 (x) v_2 (x) v_3          # [B,16] product state
  phi     = M @ psi        # M = fixed 16x16 matrix of the CNOT/RY circuit
  out     = (phi*phi)^T P + post_b  # P[i,c] = sum_w post_w[c,w] * z_w(i)

Device strategy (pure data parallel over 8 cores, 8192 samples each):
  - x is downcast to bf16 AND transposed on host, so the device streams
    xT with plain, fully-contiguous DMA (4KB descriptor lines) at full
    HBM rate -- no DMA-xbar transposes anywhere in the kernel.
  - pre-matmul: lhsT = tiny pre_w chunk [128d, 4] (stationary), rhs = xT
    block [128d, 512 samples] streaming at 1 col/cycle bf16, accumulated
    over 4 d-chunks into PSUM; two groups share a [4, 1024] PSUM tile so
    one fused bias+tanh (ScalarE) covers 1024 samples.
  - back to sample-major via tiny PE transposes: t4 blocks [4, 128] ->
    thT PSUM [128, 4] (contraction k=4, so no pad rows are ever touched);
    they pipeline at ~32ns apiece on the PE exec queue, batched 16 tiles
    per PSUM tile with one DVE copy per 2048-sample chunk.
  - trig on ScalarE: 2x Sin with scale/bias folding cos, outputs packed
    as [cos(t,w) | sin(t,w)] so the kron products use x-strided APs.
  - psi built with 3 broadcast-AP vector multiplies (bf16 out).
  - quantum circuit in bf16: PE transpose psi -> [16 comps x 8 tiles,
    samples], then block-diagonal M (16x16 circuit matrix) and P
    (measurement x post_w) matmuls with 256-col moving operands.
  - square and +post_b run on DVE; per-chunk output stores overlap.
"""

import os
import sys

for _p in ("/opt/trn_rl_repo",):
    if os.path.isdir(_p) and _p not in sys.path:
        sys.path.insert(0, _p)

import math
import numpy as np
import ml_dtypes
from contextlib import ExitStack

import concourse.bass as bass
import concourse.bacc as bacc
import concourse.mybir as mybir
from concourse.tile import TileContext
from concourse.bass_utils import run_bass_kernel_spmd

F32 = mybir.dt.float32
BF16 = mybir.dt.bfloat16
AF = mybir.ActivationFunctionType
PI4 = math.pi / 4.0

N_CORES = 8
B_FULL, D, C = 65536, 512, 10
B = B_FULL // N_CORES          # 8192 samples per core
N_QUBITS, Q_DEPTH = 4, 6
TILES = B // 128               # 64 sample tiles of 128
BLOCKS = 4                     # x load blocks of 2048 samples (4KB lines)
CHUNKS = 4                     # phase-2 chunks of 2048 samples (16 tiles)

# packed bf16 const block column offsets
CB_PREWT = 0      # [128, 16]
CB_MBD = 16       # [128, 128]
CB_PBD = 144      # [128, 80]
CB_ID = 224       # [128, 128]
CB_N = 352


# ---------------------------------------------------------------- host math
def _apply_1q(state, gate, wire):
    state = np.moveaxis(state, wire, 0)
    state = np.tensordot(gate, state, axes=((1,), (0,)))
    return np.moveaxis(state, 0, wire)


def _apply_cnot(state, ctrl, tgt):
    state = np.moveaxis(state, (ctrl, tgt), (0, 1))
    state = np.stack([state[0], state[1][::-1]], axis=0)
    return np.moveaxis(state, (0, 1), (ctrl, tgt))


def _ry(theta):
    c, s = np.cos(theta * 0.5), np.sin(theta * 0.5)
    return np.array([[c, -s], [s, c]])


def _build_M(q_params: np.ndarray) -> np.ndarray:
    """16x16 matrix of the fixed part of the circuit (after the per-sample
    RY layer): 6 repetitions of [CNOT(0,1), CNOT(2,3), CNOT(1,2), RY layer]."""
    qw = np.asarray(q_params, np.float64).reshape(Q_DEPTH, N_QUBITS)
    M = np.zeros((16, 16), np.float64)
    for i in range(16):
        state = np.zeros(16, np.float64)
        state[i] = 1.0
        state = state.reshape((2,) * N_QUBITS)
        for k in range(Q_DEPTH):
            for a in range(0, N_QUBITS - 1, 2):
                state = _apply_cnot(state, a, a + 1)
            for a in range(1, N_QUBITS - 1, 2):
                state = _apply_cnot(state, a, a + 1)
            for w in range(N_QUBITS):
                state = _apply_1q(state, _ry(qw[k, w]), w)
        M[:, i] = state.reshape(16)
    return M


def _build_P(post_w: np.ndarray) -> np.ndarray:
    """P[i, c] = sum_w post_w[c, w] * z_w(i), where z_w(i) flips sign with
    bit (3-w) of the state index i (axis 0 of the state = qubit 0)."""
    post_w = np.asarray(post_w, np.float64)
    i = np.arange(16)
    z = np.stack([1.0 - 2.0 * ((i >> (3 - w)) & 1) for w in range(N_QUBITS)], 1)
    return z @ post_w.T  # [16, 10]


# ---------------------------------------------------------------- bass build
def build_nc(sim_compat: bool = False) -> bass.Bass:
    # Bacc (not raw Bass): its finalize() runs generate_event_semaphores,
    # which splits multi-semaphore waits to satisfy the TRN2 one-wait-per-
    # instruction ISA limit.
    nc = bacc.Bacc(None)
    xt = nc.dram_tensor("xt", [D, B], BF16, kind="ExternalInput")
    cbf = nc.dram_tensor("cbf", [128, CB_N], BF16, kind="ExternalInput")
    cf32 = nc.dram_tensor("cf32", [128, 4], F32, kind="ExternalInput")
    # transposed on device: out[tile, class, sample-in-tile]; host flips back
    out = nc.dram_tensor("out", [TILES, C, 128], F32, kind="ExternalOutput")

    with ExitStack() as ctx:
        tc = ctx.enter_context(TileContext(nc))
        consts = ctx.enter_context(tc.tile_pool(name="consts", bufs=1))
        # all x block tiles stay resident (8 MB)
        xt_pool = ctx.enter_context(tc.tile_pool(name="xt", bufs=20))
        work = ctx.enter_context(tc.tile_pool(name="work", bufs=2))
        ps_po = ctx.enter_context(tc.tile_pool(name="ps_po", space="PSUM", bufs=2))
        ps_th = ctx.enter_context(tc.tile_pool(name="ps_th", space="PSUM", bufs=1))
        ps_pt = ctx.enter_context(tc.tile_pool(name="ps_pt", space="PSUM", bufs=1))
        ps_mm = ctx.enter_context(tc.tile_pool(name="ps_mm", space="PSUM", bufs=2))

        # consts go on the sync queue BEFORE the x loads so their (tiny)
        # transfers land first and the PE warmup can start early
        cbf_sb = consts.tile([128, CB_N], BF16)
        nc.sync.dma_start(cbf_sb, cbf[:, :])
        cf32_sb = consts.tile([128, 4], F32)
        nc.sync.dma_start(cf32_sb, cf32[:, :])

        pre_wt_sb = cbf_sb[:, CB_PREWT:CB_PREWT + 16]
        mbd_sb = cbf_sb[:, CB_MBD:CB_MBD + 128]
        pbd_sb = cbf_sb[:, CB_PBD:CB_PBD + 80]
        id_sb = cbf_sb[:, CB_ID:CB_ID + 128]
        id4_sb = cbf_sb[0:4, CB_ID:CB_ID + 4]
        pre_b_sb = cf32_sb[0:4, 0:1]
        pb80_sb = cf32_sb[0:80, 1:2]
        trigb_sb = cf32_sb[:, 2:4]

        # tanh staging, component-major: t4[f, b] = tanh(pre_out[b, f])
        t4_sb = consts.tile([4, B], BF16)
        # transposed output staging [80 = 8 tiles x 10 classes, 1024]
        out2_sb = consts.tile([80, 256 * CHUNKS], F32)

        # pin the activation table to silu_and_others once: it is the only
        # table containing tanh+sin+square+identity together, so no further
        # table loads happen. (CoreSim can't evaluate Silu; the sim build
        # substitutes Tanh -- the value is unused either way.)
        silu_sb = consts.tile([128, 1], F32)
        nc.scalar.activation(silu_sb, cf32_sb[:, 0:1],
                             AF.Tanh if sim_compat else AF.Silu)

        # ---- all x loads up front: plain contiguous DMA on the SP queue,
        # one per (d-chunk, 2048-sample block) with 4KB descriptor lines
        xtt = []  # xtt[b][k] = [128 (d of chunk k), 2048 samples]
        for b in range(BLOCKS):
            tiles_k = []
            for k in range(4):
                t = xt_pool.tile([128, 2048], BF16, name=f"xt{b}_{k}", tag="xt")
                nc.sync.dma_start(
                    t, xt[128 * k:128 * (k + 1), 2048 * b:2048 * (b + 1)])
                tiles_k.append(t)
            xtt.append(tiles_k)

        # PE p-state warmup while the first loads are in flight (results
        # unused; depends only on the cbf const load, which lands first)
        for w in range(4):
            wt = ps_pt.tile([128, 128], BF16, name=f"warm{w}", tag="pt")
            nc.tensor.transpose(wt, id_sb, id_sb)

        def phase1(b):
            # pre-net: two 512-groups share one [4,1024] PSUM tile (two
            # accumulation groups in adjacent banks) so a single fused
            # bias+tanh (ScalarE) covers 1024 samples
            for uu in range(2):
                po = ps_po.tile([4, 1024], F32, name="po", tag="po")
                for j in range(2):
                    gl = 2 * uu + j
                    for k in range(4):
                        nc.tensor.matmul(
                            po[:, 512 * j:512 * (j + 1)],
                            lhsT=pre_wt_sb[:, 4 * k:4 * (k + 1)],
                            rhs=xtt[b][k][:, 512 * gl:512 * (gl + 1)],
                            start=(k == 0), stop=(k == 3))
                nc.scalar.activation(
                    t4_sb[:, 2048 * b + 1024 * uu:2048 * b + 1024 * (uu + 1)],
                    po, AF.Tanh, bias=pre_b_sb[:, :])

        def phase2(T0, nt):
            # chunk = nt sample-tiles starting at tile T0 (nt = 8 or 16).
            # sample-major angles via nt tiny PE transposes (k=4
            # contraction, so only the 4 real rows of t4 are read); the
            # Sin ops read the batched transpose tile straight from PSUM
            nh = nt // 8
            thT_ps = ps_th.tile([128, 4 * nt], BF16, name="thT_ps", tag="th")
            for t in range(nt):
                T = T0 + t
                nc.tensor.transpose(
                    thT_ps[:, 4 * t:4 * (t + 1)],
                    t4_sb[:, 128 * T:128 * (T + 1)], id4_sb)
            # quick DVE copy decouples the PSUM slot from the ACT chain
            th4 = work.tile([128, 4 * nt], BF16, name="th4", tag="th4")
            nc.vector.tensor_copy(th4, thT_ps)
            th3 = th4[:, :].rearrange("p (t i) -> p t i", i=4)
            # cs2 packed: cols 0:4nt = cos(theta)[t,w], 4nt:8nt = sin[t,w]
            # cos = sin(pi/4*tanh + 3pi/4); sin = sin(.. + pi/4)
            cs2 = work.tile([128, 8 * nt], F32, name="cs2", tag="cs2")
            nc.scalar.activation(
                cs2[:, 0:4 * nt].rearrange("p (t w) -> p t w", w=4), th3,
                AF.Sin, bias=trigb_sb[:, 0:1], scale=PI4)
            nc.scalar.activation(
                cs2[:, 4 * nt:8 * nt].rearrange("p (t w) -> p t w", w=4), th3,
                AF.Sin, bias=trigb_sb[:, 1:2], scale=PI4)
            ctxw = cs2[:, :].rearrange("p (x t w) -> p t x w", x=2, w=4)
            ctwx = cs2[:, :].rearrange("p (x t w) -> p t w x", x=2, w=4)
            v01 = work.tile([128, 4 * nt], F32, name="v01", tag="v01")
            v23 = work.tile([128, 4 * nt], F32, name="v23", tag="v23")
            nc.vector.tensor_tensor(
                out=v01[:, :].rearrange("p (t a b) -> p t a b", a=2, b=2),
                in0=ctxw[:, :, :, 0:1].broadcast_to((128, nt, 2, 2)),
                in1=ctwx[:, :, 1:2, :].broadcast_to((128, nt, 2, 2)),
                op=mybir.AluOpType.mult)
            nc.vector.tensor_tensor(
                out=v23[:, :].rearrange("p (t a b) -> p t a b", a=2, b=2),
                in0=ctxw[:, :, :, 2:3].broadcast_to((128, nt, 2, 2)),
                in1=ctwx[:, :, 3:4, :].broadcast_to((128, nt, 2, 2)),
                op=mybir.AluOpType.mult)
            psi = work.tile([128, 16 * nt], BF16, name="psi", tag="psi")
            nc.vector.tensor_tensor(
                out=psi[:, :].rearrange("p (t a b) -> p t a b", a=4, b=4),
                in0=v01[:, :].rearrange("p (t i) -> p t i", i=4)
                    .unsqueeze(3).broadcast_to((128, nt, 4, 4)),
                in1=v23[:, :].rearrange("p (t i) -> p t i", i=4)
                    .unsqueeze(2).broadcast_to((128, nt, 4, 4)),
                op=mybir.AluOpType.mult)

            # quantum circuit, all bf16: psi^T per h-half, then block-diag
            # M and P matmuls over all halves at once
            psiT_ps = ps_pt.tile([128, 128 * nh], BF16, name="psiT_ps",
                                 tag="pt")
            for h in range(nh):
                nc.tensor.transpose(
                    psiT_ps[:, 128 * h:128 * (h + 1)],
                    psi[:, 128 * h:128 * (h + 1)], id_sb)
            psiT = work.tile([128, 128 * nh], BF16, name="psiT", tag="psiT")
            nc.vector.tensor_copy(psiT, psiT_ps)
            phi_ps = ps_mm.tile([128, 128 * nh], F32, name="phi_ps", tag="mm")
            nc.tensor.matmul(phi_ps, lhsT=mbd_sb, rhs=psiT,
                             start=True, stop=True)
            phi2 = work.tile([128, 128 * nh], BF16, name="phi2", tag="phi2")
            nc.scalar.activation(phi2, phi_ps, AF.Square)
            o10_ps = ps_mm.tile([80, 128 * nh], F32, name="o10_ps", tag="mm")
            nc.tensor.matmul(o10_ps, lhsT=pbd_sb, rhs=phi2,
                             start=True, stop=True)
            # bias-add on DVE (per-partition scalar operand) into the
            # transposed output staging tile
            nc.vector.tensor_scalar(
                out2_sb[:, 16 * T0:16 * T0 + 128 * nh], o10_ps,
                pb80_sb[:, :], None, mybir.AluOpType.add)

            # per-chunk store (plain, on the sync queue behind the loads):
            # out[T0 + 8h + t, cls, j] = out2_sb[10t + cls, 16*T0 + 128h + j]
            nc.sync.dma_start(
                out[T0:T0 + nt, :, :]
                    .rearrange("(h t) c j -> (t c) h j", h=nh),
                out2_sb[:, 16 * T0:16 * T0 + 128 * nh]
                    .rearrange("p (h j) -> p h j", h=nh))

        for b in range(BLOCKS):
            phase1(b)
            phase2(16 * b, 16)

    nc.finalize()  # bacc: register alloc + event-semaphore wait splitting
    return nc


_NC_CACHE: dict = {}


def _get_nc() -> bass.Bass:
    if "nc" not in _NC_CACHE:
        _NC_CACHE["nc"] = build_nc()
    return _NC_CACHE["nc"]


def make_in_maps(inputs: dict) -> list:
    x = np.asarray(inputs["input_features"], np.float32)
    pre_w = np.asarray(inputs["pre_w"], np.float32)
    pre_b = np.asarray(inputs["pre_b"], np.float32)
    q_params = np.asarray(inputs["q_params"], np.float32)
    post_w = np.asarray(inputs["post_w"], np.float32)
    post_b = np.asarray(inputs["post_b"], np.float32)

    M = _build_M(q_params)
    P = _build_P(post_w)
    mbd = np.zeros((128, 128), np.float32)
    pbd = np.zeros((128, 80), np.float32)
    for t in range(8):
        mbd[16 * t:16 * (t + 1), 16 * t:16 * (t + 1)] = M.T
        pbd[16 * t:16 * (t + 1), 10 * t:10 * (t + 1)] = P
    # pre_wt_sb[p, 4k+f] = pre_w[f, 128k+p]
    pre_wt = np.ascontiguousarray(
        pre_w.T.reshape(4, 128, 4).transpose(1, 0, 2).reshape(128, 16))

    cbf = np.zeros((128, CB_N), np.float32)
    cbf[:, CB_PREWT:CB_PREWT + 16] = pre_wt
    cbf[:, CB_MBD:CB_MBD + 128] = mbd
    cbf[:, CB_PBD:CB_PBD + 80] = pbd
    cbf[:, CB_ID:CB_ID + 128] = np.eye(128, dtype=np.float32)
    cbf = cbf.astype(ml_dtypes.bfloat16)

    cf32 = np.zeros((128, 4), np.float32)
    cf32[0:4, 0] = pre_b
    cf32[0:80, 1] = np.tile(post_b, 8)
    cf32[:, 2] = 3.0 * PI4
    cf32[:, 3] = PI4

    xb = x.astype(ml_dtypes.bfloat16)
    consts = dict(cbf=cbf, cf32=cf32)
    return [dict(xt=np.ascontiguousarray(xb[B * i:B * (i + 1)].T), **consts)
            for i in range(N_CORES)]


def unpack_out(dev_out: np.ndarray) -> np.ndarray:
    """[TILES, C, 128] device layout -> [B, C]."""
    return dev_out.transpose(0, 2, 1).reshape(B, C)


def run_on_device(inputs: dict, **kwargs):
    """Returns (full_output, BassKernelResults)."""
    nc = _get_nc()
    in_maps = make_in_maps(inputs)
    res = run_bass_kernel_spmd(nc, in_maps, core_ids=list(range(N_CORES)),
                               **kwargs)
    full = np.concatenate(
        [unpack_out(res.results[i]["out"]) for i in range(N_CORES)], 0)
    return np.ascontiguousarray(full, dtype=np.float32), res


def kernel(**inputs) -> np.ndarray:
    out, _ = run_on_device(inputs)
    return out
